# revision 67
# baseline (speedup 1.0000x reference)
"""Trainium2 Bass kernel for nn_AttentionBlock_33724083208839 (sparse_attention).

Data-parallel over batch (8 batches -> 8 cores). Per core:
  1. chunked x load (all DMAs issued upfront), PE transpose -> xT f32;
     K projected in exact f32 (feeds selection), Q in f32; both also copied
     to bf16 (kTb via ACT, qTb via DVE) for the attention matmuls; V in f32
     -> bf16 [V|1] tiles.
  2. K_reduce via the exact CVaR identity sum_top_l = l*t + sum(relu(x-t)),
     t from Gaussian quantile + one Newton step on the exact count (f32 DVE).
  3. query selection: sqk = x @ (Wq @ K_reduce) exactly on PE; threshold =
     LQ-th largest of sqk via two full 128-ary counting passes on a
     partition-replicated copy, then sparse_gather (GPSIMD) compacts the
     ~10 in-interval values (num_found-masked: HW leaves garbage pads) and
     three cheap passes on the compacted set finish to f32 resolution.
  4. attention for all 4096 queries, 512-query slabs with a 1-slab lag and
     triple-buffered P^T tiles (so scores of slab s never wait on AV of
     slab s-2 releasing a buffer); x is loaded through a 16-chunk rolling
     window:
     scores^T on PE (bf16, 2-ktile PSUM strips) -> exp split between ACT
     (exact, scale=1/8, own 2-slot strip ring) and DVE (Schraudolph bitcast
     exp int16(A*s+B) -> bf16, own strip ring) -> P^T bf16 -> reversed AV:
     lhsT = P^T tile (stationary), rhs = [V|1] (65 moving cols) accumulated
     over 32 k-tiles, interleaved into the score-group stream, so outputs
     land directly in [query-partition, dv] layout; normalize by the
     ones-column denominator (DVE), blend non-selected rows to meanV
     (copy_predicated), DMA out per 128-query chunk.
"""
import sys

sys.path.insert(0, "/opt/trn_rl_repo")

import math
from statistics import NormalDist

import numpy as np

import concourse.bacc as bacc
import concourse.bass as bass
import concourse.bass_isa as bass_isa
import concourse.mybir as mybir
from concourse.tile import TileContext
from concourse.masks import make_identity
from concourse.bass_utils import run_bass_kernel_spmd

B, L, D = 8, 4096, 64
LQ = int((1.0 - 0.33) * L)  # 2744
PART = 128
NT = L // PART
NS = L // 512
N_CORES = 8

QFRAC = 1.0 - LQ / L
Z = NormalDist().inv_cdf(QFRAC)
PHI = math.exp(-Z * Z / 2.0) / math.sqrt(2.0 * math.pi)

f32 = mybir.dt.float32
f32r = mybir.dt.float32r
bf16 = mybir.dt.bfloat16
u8 = mybir.dt.uint8
i16 = mybir.dt.int16
i32 = mybir.dt.int32
AF = mybir.ActivationFunctionType
OP = mybir.AluOpType

N_PASS = 5
BOUND = 512.0

# Schraudolph exp for bf16 bit patterns: bf16_bits(exp(s/8)) ~= A*s + B.
# A = 128*log2(e)/8; B centers the piecewise-linear sawtooth (mean-unbiased)
# and adds +0.5 to compensate truncation in the float->int16 convert.
SCH_A = 128.0 * math.log2(math.e) / 8.0
SCH_B = 16256.0 + 0.5 - 128.0 * math.log2(1.0407)

GROUPS = [(g, 2) for g in range(0, NT, 2)]

# exp-engine split: selection runs on GPSIMD, so the DVE takes a fixed share
# of the exp strips (Schraudolph), spread through the slab so both engines
# drain the strip ring concurrently.
DVE_GROUPS = {2, 5, 8, 11, 14}
DVE_FROM_SLAB = 1
DVE_GROUPS_S0 = set()


def build(debug: bool = False):
    nc = bacc.Bacc("TRN2")
    x = nc.dram_tensor("x", [L, D], f32, kind="ExternalInput")
    wq = nc.dram_tensor("Wq", [D, D], f32, kind="ExternalInput")
    wk = nc.dram_tensor("Wk", [D, D], f32, kind="ExternalInput")
    wv = nc.dram_tensor("Wv", [D, D], f32, kind="ExternalInput")
    out = nc.dram_tensor("out", [L, D], f32, kind="ExternalOutput")
    scr_row = nc.dram_tensor("scr_row", [1, L], f32, kind="Internal")
    scr_cmp = nc.dram_tensor("scr_cmp", [1, 1024], f32, kind="Internal")
    scr_chi = nc.dram_tensor("scr_chi", [1, 16], f32, kind="Internal")
    dbg = {}
    if debug:
        for name, shape in [
            ("dbg_kr", [D, 1]), ("dbg_thr", [PART, 1]), ("dbg_sqk", [PART, NT]),
            ("dbg_mask", [PART, NT]), ("dbg_tk", [PART, 1]), ("dbg_cnt", [PART, 1]),
            ("dbg_sel", [PART, 8]), ("dbg_valrep", [PART, 64]),
        ]:
            dbg[name] = nc.dram_tensor(name, shape, f32, kind="ExternalOutput")

    x_re = x[:].rearrange("(c p) d -> p c d", p=PART)
    out_re = out[:].rearrange("(c p) d -> p c d", p=PART)

    with TileContext(nc) as tc, \
         tc.tile_pool(name="cst", bufs=1) as cst, \
         tc.tile_pool(name="big", bufs=1) as big, \
         tc.tile_pool(name="sc", bufs=1) as sc, \
         tc.tile_pool(name="mn", bufs=2) as mn:

        # ---- warm the exp activation table immediately ----
        warm = cst.tile([1, 8], f32)
        nc.vector.memset(warm[:], 0.0)
        warm2 = cst.tile([1, 8], f32)
        nc.scalar.activation(out=warm2[:], in_=warm[:], func=AF.Exp)

        # ---- constants ----
        ident = cst.tile([PART, PART], f32)
        make_identity(nc, ident[:])
        onesb = cst.tile([PART, 1], bf16)
        nc.vector.memset(onesb[:], 1.0)
        ones1x128 = cst.tile([1, PART], f32)
        nc.vector.memset(ones1x128[:], 1.0)
        iotc_i = cst.tile([16, 64], i32)
        nc.gpsimd.iota(iotc_i[:], pattern=[[16, 64]], base=0, channel_multiplier=1)
        iotc = cst.tile([16, 64], f32)
        nc.vector.tensor_copy(iotc[:], iotc_i[:])
        pidx1i = cst.tile([PART, 1], i32)
        nc.gpsimd.iota(pidx1i[:], pattern=[[1, 1]], base=1, channel_multiplier=1)
        pidx1 = cst.tile([PART, 1], f32)
        nc.vector.tensor_copy(pidx1[:], pidx1i[:])

        # ---- persistent tensors ----
        x_sb = big.tile([PART, 16, D], f32)
        xT32 = big.tile([D, L], f32)
        qTb = big.tile([D, L], bf16)
        kT32 = big.tile([D, L], f32)
        kTb = big.tile([D, L], bf16)
        vp = big.tile([PART, NT, D + 1], bf16)
        pt_a = big.tile([PART, NT, 512], bf16)
        pt_b = big.tile([PART, NT, 512], bf16)
        pt_c = big.tile([PART, NT, 512], bf16)
        res = big.tile([PART, NT, D], f32)
        mvf = big.tile([PART, D], f32)
        mask = big.tile([PART, NT], f32)
        inv_u8 = big.tile([PART, NT], u8)
        sqk = big.tile([PART, NT], f32)
        kr = big.tile([D, 1], f32)
        wvec = big.tile([D, 1], f32)
        sqk_rep = big.tile([PART, L], f32)
        cmp_rep = big.tile([PART, L], bf16)
        sqk16 = big.tile([16, 256], f32)
        valrep = big.tile([PART, 1024], f32)
        cmpc = big.tile([PART, 1024], bf16)

        # weights
        wq_s = cst.tile([D, D], f32)
        wk_s = cst.tile([D, D], f32)
        wv_s = cst.tile([D, D], f32)
        nc.sync.dma_start(out=wq_s[:], in_=wq[:])
        nc.sync.dma_start(out=wk_s[:], in_=wk[:])
        nc.sync.dma_start(out=wv_s[:], in_=wv[:])

        # =============== phase 1: load / project / slab-0 scores+exp ===============
        with tc.tile_pool(name="ps_xv", bufs=2, space="PSUM") as ps_xv, \
             tc.tile_pool(name="ps_pj", bufs=2, space="PSUM") as ps_pj, \
             tc.tile_pool(name="ps_s0", bufs=2, space="PSUM") as ps_s0:

            def load_tiles(c0, c1):
                for c in range(c0, c1):
                    pxt = ps_xv.tile([PART, PART], f32, tag="xv")
                    nc.tensor.transpose(out=pxt[0:D, :], in_=x_sb[:, c % 16, :],
                                        identity=ident[:])
                    nc.vector.tensor_copy(xT32[:, PART * c:PART * (c + 1)], pxt[0:D, :])
                if c0 + 16 < NT:
                    m = c0 % 16
                    nc.sync.dma_start(out=x_sb[:, m:m + (c1 - c0), :],
                                      in_=x_re[:, c0 + 16:c1 + 16, :])

            def proj_slab(s):
                sl = slice(512 * s, 512 * (s + 1))
                pk = ps_pj.tile([D, 512], f32, tag="pj")
                for h in range(2):
                    hs = slice(512 * s + 256 * h, 512 * s + 256 * (h + 1))
                    nc.tensor.matmul(out=pk[:, 256 * h:256 * (h + 1)], lhsT=wk_s[:],
                                     rhs=xT32[:, hs], start=True, stop=True)
                    nc.vector.tensor_copy(kT32[:, hs], pk[:, 256 * h:256 * (h + 1)])
                    nc.scalar.copy(kTb[:, hs], pk[:, 256 * h:256 * (h + 1)])
                pq = ps_pj.tile([D, 512], f32, tag="pj")
                nc.tensor.matmul(out=pq[:], lhsT=wq_s[:], rhs=xT32[:, sl],
                                 start=True, stop=True)
                nc.vector.tensor_copy(qTb[:, sl], pq[:])

            def sg0(gi):
                g0, glen = GROUPS[gi]
                strip = ps_s0.tile([PART, 2, 512], f32, tag="s0")
                for i in range(glen):
                    j = g0 + i
                    nc.tensor.matmul(out=strip[:, i, :],
                                     lhsT=kTb[:, PART * j:PART * (j + 1)],
                                     rhs=qTb[:, 0:512], start=True, stop=True)
                if gi in DVE_GROUPS_S0:  # slab 0
                    nc.vector.tensor_scalar(
                        out=pt_a[:, g0:g0 + glen, :].bitcast(i16),
                        in0=strip[:, 0:glen, :], scalar1=SCH_A, scalar2=SCH_B,
                        op0=OP.mult, op1=OP.add)
                else:
                    nc.scalar.activation(out=pt_a[:, g0:g0 + glen, :],
                                         in_=strip[:, 0:glen, :], func=AF.Exp, scale=0.125)

            def proj_v(c0, c1):
                for c in range(c0, c1):
                    pv = ps_xv.tile([PART, PART], f32, tag="xv")
                    nc.tensor.matmul(out=pv[:, 0:D],
                                     lhsT=xT32[:, PART * c:PART * (c + 1)],
                                     rhs=wv_s[:], start=True, stop=True)
                    nc.scalar.copy(vp[:, c, 0:D], pv[:, 0:D])

            for c0 in range(0, 16, 4):
                nc.sync.dma_start(out=x_sb[:, c0:c0 + 4, :], in_=x_re[:, c0:c0 + 4, :])
            load_tiles(0, 2)
            load_tiles(2, 4)
            proj_slab(0)
            proj_v(0, 4)
            sg0(0); sg0(1)
            load_tiles(4, 8)
            proj_slab(1)
            proj_v(4, 8)
            sg0(2); sg0(3)
            load_tiles(8, 12)
            load_tiles(12, 16)
            proj_slab(2); proj_v(8, 12); sg0(4); sg0(5)
            proj_slab(3); proj_v(12, 16); sg0(6); sg0(7)
            load_tiles(16, 20)
            load_tiles(20, 24)
            proj_slab(4); proj_v(16, 20); sg0(8); sg0(9)
            proj_slab(5); proj_v(20, 24); sg0(10); sg0(11)
            load_tiles(24, 28)
            load_tiles(28, 32)
            proj_slab(6); proj_v(24, 28); sg0(12); sg0(13)
            proj_slab(7); proj_v(28, 32)
            nc.vector.memset(vp[:, :, D:D + 1], 1.0)
            sg0(14)
            sg0(15)

        # ---- K_reduce (DVE only; channel = partition of kT32) ----
        bstats = sc.tile([D, 8, 6], f32, tag="bstats")
        for a in range(8):
            nc.vector.bn_stats(bstats[:, a, :], kT32[:, 512 * a:512 * (a + 1)])
        aggr = sc.tile([D, 2], f32, tag="aggr")
        nc.vector.bn_aggr(aggr[:], bstats[:])
        sig = sc.tile([D, 1], f32, tag="sig")
        nc.vector.memset(sig[:], 1.0)
        for _ in range(4):
            rs = sc.tile([D, 1], f32, tag="rs")
            nc.vector.reciprocal(rs[:], sig[:])
            nc.vector.tensor_tensor(out=rs[:], in0=rs[:], in1=aggr[:, 1:2], op=OP.mult)
            nc.vector.tensor_tensor(out=rs[:], in0=rs[:], in1=sig[:], op=OP.add)
            nc.vector.tensor_scalar_mul(sig[:], rs[:], 0.5)
        tk = sc.tile([D, 1], f32, tag="tk")
        nc.vector.tensor_scalar(out=tk[:], in0=sig[:], scalar1=float(Z),
                                scalar2=None, op0=OP.mult)
        nc.vector.tensor_tensor(out=tk[:], in0=tk[:], in1=aggr[:, 0:1], op=OP.add)
        cnt_c = sc.tile([D, 1], f32, tag="cnt_c")
        nc.vector.tensor_scalar(out=sqk_rep[0:D, :], in0=kT32[:], scalar1=tk[:, 0:1],
                                scalar2=None, op0=OP.is_gt, op1=OP.add,
                                accum_out=cnt_c[:])
        adj = sc.tile([D, 1], f32, tag="adj")
        nc.vector.tensor_scalar(out=adj[:], in0=cnt_c[:], scalar1=float(-LQ),
                                scalar2=1.0 / (L * PHI), op0=OP.add, op1=OP.mult)
        nc.vector.tensor_tensor(out=adj[:], in0=adj[:], in1=sig[:], op=OP.mult)
        t1 = sc.tile([D, 1], f32, tag="t1")
        nc.vector.tensor_tensor(out=t1[:], in0=tk[:], in1=adj[:], op=OP.add)
        nc.vector.tensor_scalar(out=sqk_rep[0:D, :], in0=kT32[:], scalar1=t1[:, 0:1],
                                scalar2=0.0, op0=OP.subtract, op1=OP.max)
        s1c = sc.tile([D, 1], f32, tag="s1c")
        nc.vector.tensor_reduce(out=s1c[:], in_=sqk_rep[0:D, :], axis=mybir.AxisListType.X,
                                op=OP.add)
        nc.vector.tensor_scalar(out=kr[:], in0=s1c[:], scalar1=1.0 / LQ,
                                scalar2=None, op0=OP.mult)
        nc.vector.tensor_tensor(out=kr[:], in0=kr[:], in1=t1[:], op=OP.add)

        emit_kred_a()

        # =============== phase 2: attention + selection ===============
        with tc.tile_pool(name="ps_strip", bufs=2, space="PSUM") as ps_strip, \
             tc.tile_pool(name="ps_dstrip", bufs=1, space="PSUM") as ps_dstrip, \
             tc.tile_pool(name="ps_av", bufs=2, space="PSUM") as ps_av:
            def mis_tile():
                return ps_av.tile([PART, PART], f32, tag="av", name="avm")

            def pt_of(s):
                return (pt_a, pt_b, pt_c)[s % 3]

            # ---- selection PE bits: wvec, sqk, replicate sqk ----
            pwt = mis_tile()
            nc.tensor.transpose(out=pwt[0:D, 0:D], in_=wq_s[:], identity=ident[0:D, 0:D])
            wqT = sc.tile([D, D], f32, tag="wqT")
            nc.vector.tensor_copy(wqT[:], pwt[0:D, 0:D])
            pw = mis_tile()
            nc.tensor.matmul(out=pw[0:D, 0:1], lhsT=wqT[:], rhs=kr[:],
                             start=True, stop=True)
            nc.vector.tensor_copy(wvec[:], pw[0:D, 0:1])
            psq = mis_tile()
            for c in range(NT):
                nc.tensor.matmul(out=psq[:, c:c + 1],
                                 lhsT=xT32[:, PART * c:PART * (c + 1)],
                                 rhs=wvec[:], start=True, stop=True)
            nc.vector.tensor_copy(sqk[:], psq[:, 0:NT])

            psqT = mis_tile()
            nc.tensor.transpose(out=psqT[0:NT, 0:PART], in_=sqk[:], identity=ident[:])
            sqkT = sc.tile([NT, PART], f32, tag="sqkT")
            nc.vector.tensor_copy(sqkT[:], psqT[0:NT, 0:PART])
            nc.sync.dma_start(out=scr_row[:], in_=sqkT[:])
            nc.sync.dma_start(out=sqk_rep[:], in_=scr_row[:].to_broadcast([PART, L]))

            if debug:
                nc.sync.dma_start(out=dbg["dbg_kr"][:], in_=kr[:])
                nc.sync.dma_start(out=dbg["dbg_sqk"][:], in_=sqk[:])
                nc.sync.dma_start(out=dbg["dbg_tk"][0:D, :], in_=t1[:])

            # ---- meanV on PE ----
            pmv = mis_tile()
            for c in range(NT):
                nc.tensor.matmul(out=pmv[0:D + 1, 0:1], lhsT=vp[:, c, :], rhs=onesb[:],
                                 start=(c == 0), stop=(c == NT - 1))
            mv_col = sc.tile([D, 1], f32, tag="mv_col")
            nc.vector.tensor_scalar_mul(mv_col[:], pmv[0:D, 0:1], 1.0 / L)
            pmvT = mis_tile()
            nc.tensor.transpose(out=pmvT[0:1, 0:D], in_=mv_col[:],
                                identity=ident[0:D, 0:D])
            mv_row = sc.tile([1, D], f32, tag="mv_row")
            nc.vector.tensor_copy(mv_row[:], pmvT[0:1, 0:D])
            pmvF = mis_tile()
            nc.tensor.matmul(out=pmvF[:, 0:D], lhsT=ones1x128[:], rhs=mv_row[:],
                             start=True, stop=True)
            nc.vector.tensor_copy(mvf[:], pmvF[:, 0:D])

            # ---- selection: two full 128-ary passes on the replicated sqk,
            # then sparse_gather compacts the ~10 in-interval values and three
            # cheap passes on the compacted set finish to f32 resolution ----
            sel_state = {}
            DLT1 = 2.0 * BOUND / 129.0
            DLT2 = DLT1 / 129.0        # interval width after pass 2
            DLTC = [DLT2 / 129.0, DLT2 / 129.0 ** 2, DLT2 / 129.0 ** 3]

            def sel_pass_init():
                lo = mn.tile([PART, 1], f32, tag="lo_a")
                nc.vector.memset(lo[:], -BOUND)
                sel_state["lo"] = lo

            def sel_pass_full(it):
                # thresholds t_p = lo + p*dlt; count(sqk > t_p) per partition
                lo = sel_state["lo"]
                dlt = DLT1 if it == 0 else DLT2
                tvec = mn.tile([PART, 1], f32, tag=f"tv{it % 2}")
                nc.vector.tensor_scalar(out=tvec[:], in0=pidx1[:], scalar1=float(dlt),
                                        scalar2=None, op0=OP.mult)
                nc.vector.tensor_tensor(out=tvec[:], in0=tvec[:], in1=lo[:], op=OP.add)
                cntq = mn.tile([PART, 1], f32, tag="cntq")
                nc.vector.tensor_scalar(out=cmp_rep[:], in0=sqk_rep[:],
                                        scalar1=tvec[:, 0:1], scalar2=None,
                                        op0=OP.is_gt, op1=OP.add, accum_out=cntq[:])
                sel = mn.tile([PART, 1], f32, tag="sel")
                nc.vector.tensor_scalar(out=sel[:], in0=cntq[:], scalar1=float(LQ),
                                        scalar2=None, op0=OP.is_ge)
                jsr = mn.tile([PART, 1], f32, tag="jsr")
                nc.gpsimd.partition_all_reduce(jsr[:], sel[:], channels=PART,
                                               reduce_op=bass_isa.ReduceOp.add)
                nlo = mn.tile([PART, 1], f32, tag=f"lo_{'b' if it % 2 == 0 else 'a'}")
                nc.vector.tensor_scalar(out=jsr[:], in0=jsr[:], scalar1=float(dlt),
                                        scalar2=None, op0=OP.mult)
                nc.vector.tensor_tensor(out=nlo[:], in0=lo[:], in1=jsr[:], op=OP.add)
                sel_state["lo"] = nlo

            def sel_compact():
                # threshold in (lo2, lo2 + DLT2]; c_hi = count(sqk > hi2) exact;
                # compact v' = sqk - lo2 for in-interval values via sparse_gather
                lo2 = sel_state["lo"]
                hi2 = mn.tile([PART, 1], f32, tag="hi2")
                nc.vector.tensor_scalar(out=hi2[:], in0=lo2[:], scalar1=float(DLT2),
                                        scalar2=None, op0=OP.add)
                nc.sync.dma_start(out=sqk16[:], in_=scr_row[0, :].rearrange(
                    "(f p) -> p f", p=16))
                # c_hi = count(sqk > hi2), counted on the [128, 32] per-query
                # tile + a channels=128 all-reduce (HW-proven path)
                j32 = sc.tile([PART, NT], f32, tag="j32")
                chi = sc.tile([PART, 1], f32, tag="chi")
                nc.vector.tensor_scalar(out=j32[:], in0=sqk[:],
                                        scalar1=hi2[:, 0:1], scalar2=None,
                                        op0=OP.is_gt, op1=OP.add, accum_out=chi[:])
                nc.gpsimd.partition_all_reduce(chi[:], chi[:], channels=PART,
                                               reduce_op=bass_isa.ReduceOp.add)
                rvec = sc.tile([PART, 1], f32, tag="rvec")
                nc.vector.tensor_scalar(out=rvec[:], in0=chi[:], scalar1=-1.0,
                                        scalar2=float(LQ), op0=OP.mult, op1=OP.add)
                sel_state["rvec"] = rvec
                # Tv = (sqk-lo2)*b + (b-1) with b = (sqk <= hi2): in-interval ->
                # positive v', others -> negative (sparse_gather keeps >= 0)
                a16 = sc.tile([16, 256], f32, tag="a16")
                nc.vector.tensor_scalar(out=a16[:], in0=sqk16[:],
                                        scalar1=lo2[0:16, 0:1], scalar2=None,
                                        op0=OP.subtract)
                b16 = sc.tile([16, 256], f32, tag="b16")
                nc.vector.tensor_scalar(out=b16[:], in0=sqk16[:],
                                        scalar1=hi2[0:16, 0:1], scalar2=None,
                                        op0=OP.is_le)
                tv16 = sc.tile([16, 256], f32, tag="tv16")
                nc.vector.tensor_tensor(out=tv16[:], in0=a16[:], in1=b16[:],
                                        op=OP.mult)
                nc.vector.tensor_scalar(out=b16[:], in0=b16[:], scalar1=1.0,
                                        scalar2=None, op0=OP.subtract)
                nc.vector.tensor_tensor(out=tv16[:], in0=tv16[:], in1=b16[:],
                                        op=OP.add)
                valc = sc.tile([16, 64], f32, tag="valc")
                nc.vector.memset(valc[:], -1.0)
                nfound = sc.tile([1, 1], mybir.dt.uint32, tag="nfound")
                nc.gpsimd.sparse_gather(valc[:], tv16[:], num_found=nfound[:])
                # HW sparse_gather leaves garbage beyond num_found: mask pads
                nf32 = sc.tile([1, 1], f32, tag="nf32")
                nc.vector.tensor_copy(nf32[:], nfound[:])
                nc.sync.dma_start(out=scr_chi[0:1, 0:1], in_=nf32[:])
                nfb = sc.tile([16, 1], f32, tag="nfb")
                nc.sync.dma_start(out=nfb[:],
                                  in_=scr_chi[0:1, 0:1].to_broadcast([16, 1]))
                vmask = sc.tile([16, 64], f32, tag="vmask")
                nc.vector.tensor_scalar(out=vmask[:], in0=iotc[:],
                                        scalar1=nfb[:, 0:1], scalar2=None,
                                        op0=OP.is_lt)
                nc.vector.tensor_tensor(out=valc[:], in0=valc[:], in1=vmask[:],
                                        op=OP.mult)
                nc.vector.tensor_scalar(out=vmask[:], in0=vmask[:], scalar1=1.0,
                                        scalar2=None, op0=OP.subtract)
                nc.vector.tensor_tensor(out=valc[:], in0=valc[:], in1=vmask[:],
                                        op=OP.add)
                nc.sync.dma_start(out=scr_cmp[0, :].rearrange("(f p) -> p f", p=16),
                                  in_=valc[:])
                nc.sync.dma_start(out=valrep[:],
                                  in_=scr_cmp[:].to_broadcast([PART, 1024]))
                loc = mn.tile([PART, 1], f32, tag="loc_a")
                nc.vector.memset(loc[:], 0.0)
                sel_state["loc"] = loc

            def sel_pass_c(it):
                loc, rvec = sel_state["loc"], sel_state["rvec"]
                dlt = DLTC[it]
                tvec = mn.tile([PART, 1], f32, tag=f"tvc{it % 2}")
                nc.vector.tensor_scalar(out=tvec[:], in0=pidx1[:], scalar1=float(dlt),
                                        scalar2=None, op0=OP.mult)
                nc.vector.tensor_tensor(out=tvec[:], in0=tvec[:], in1=loc[:], op=OP.add)
                cntq = mn.tile([PART, 1], f32, tag="cntqc")
                nc.vector.tensor_scalar(out=cmpc[:], in0=valrep[:],
                                        scalar1=tvec[:, 0:1], scalar2=None,
                                        op0=OP.is_gt, op1=OP.add, accum_out=cntq[:])
                sel = mn.tile([PART, 1], f32, tag="selc")
                nc.vector.tensor_scalar(out=sel[:], in0=cntq[:],
                                        scalar1=rvec[:, 0:1], scalar2=None,
                                        op0=OP.is_ge)
                jsr = mn.tile([PART, 1], f32, tag="jsrc")
                nc.gpsimd.partition_all_reduce(jsr[:], sel[:], channels=PART,
                                               reduce_op=bass_isa.ReduceOp.add)
                nlo = mn.tile([PART, 1], f32, tag=f"loc_{'b' if it % 2 == 0 else 'a'}")
                nc.vector.tensor_scalar(out=jsr[:], in0=jsr[:], scalar1=float(dlt),
                                        scalar2=None, op0=OP.mult)
                nc.vector.tensor_tensor(out=nlo[:], in0=loc[:], in1=jsr[:], op=OP.add)
                sel_state["loc"] = nlo

            def sel_finish():
                lo2, loc = sel_state["lo"], sel_state["loc"]
                if debug:
                    dsel = mn.tile([PART, 8], f32, tag="dsel")
                    nc.vector.tensor_copy(dsel[:, 0:1], sel_state["rvec"][:])
                    nc.vector.tensor_copy(dsel[:, 1:2], lo2[:])
                    nc.vector.tensor_copy(dsel[:, 2:3], loc[:])
                    nc.sync.dma_start(out=dbg["dbg_sel"][:], in_=dsel[:])
                    nc.sync.dma_start(out=dbg["dbg_valrep"][:], in_=valrep[:, 0:64])
                v128 = mn.tile([PART, NT], f32, tag="v128")
                nc.vector.tensor_scalar(out=v128[:], in0=sqk[:],
                                        scalar1=lo2[:, 0:1], scalar2=None,
                                        op0=OP.subtract)
                nc.vector.tensor_scalar(out=mask[:], in0=v128[:],
                                        scalar1=loc[:, 0:1], scalar2=None,
                                        op0=OP.is_gt)
                minv = mn.tile([PART, NT], f32, tag="minv")
                nc.vector.tensor_scalar(out=minv[:], in0=mask[:], scalar1=-1.0,
                                        scalar2=1.0, op0=OP.mult, op1=OP.add)
                nc.vector.tensor_copy(inv_u8[:], minv[:])
                if debug:
                    thrd = mn.tile([PART, 1], f32, tag="thrd")
                    nc.vector.tensor_tensor(out=thrd[:], in0=lo2[:], in1=loc[:],
                                            op=OP.add)
                    nc.sync.dma_start(out=dbg["dbg_mask"][:], in_=mask[:])
                    nc.sync.dma_start(out=dbg["dbg_thr"][:], in_=thrd[:])
                    cntf = mn.tile([PART, 1], f32, tag="cntf")
                    cmpf = mn.tile([PART, NT], f32, tag="cmpf")
                    nc.vector.tensor_scalar(out=cmpf[:], in0=mask[:], scalar1=1.0,
                                            scalar2=None, op0=OP.mult, op1=OP.add,
                                            accum_out=cntf[:])
                    nc.sync.dma_start(out=dbg["dbg_cnt"][:], in_=cntf[:])

            # ---- attention slab machinery ----
            def score_group(s, ptc, gi):
                g0, glen = GROUPS[gi]
                is_dve = s >= DVE_FROM_SLAB and gi in DVE_GROUPS
                if is_dve:
                    strip = ps_dstrip.tile([PART, 2, 512], f32, tag="dstrip")
                else:
                    strip = ps_strip.tile([PART, 2, 512], f32, tag="strip")
                for i in range(glen):
                    j = g0 + i
                    nc.tensor.matmul(out=strip[:, i, :],
                                     lhsT=kTb[:, PART * j:PART * (j + 1)],
                                     rhs=qTb[:, 512 * s:512 * (s + 1)],
                                     start=True, stop=True)
                if is_dve:
                    nc.vector.tensor_scalar(
                        out=ptc[:, g0:g0 + glen, :].bitcast(i16),
                        in0=strip[:, 0:glen, :], scalar1=SCH_A, scalar2=SCH_B,
                        op0=OP.mult, op1=OP.add)
                else:
                    nc.scalar.activation(out=ptc[:, g0:g0 + glen, :],
                                         in_=strip[:, 0:glen, :], func=AF.Exp,
                                         scale=0.125)

            def av_subtile(s, ptp, u):
                c = 4 * s + u
                av = mis_tile()
                for j in range(NT):
                    nc.tensor.matmul(out=av[:, 0:D + 1],
                                     lhsT=ptp[:, j, PART * u:PART * (u + 1)],
                                     rhs=vp[:, j, :],
                                     start=(j == 0), stop=(j == NT - 1))
                rec = mn.tile([PART, 1], f32, tag="rec")
                nc.vector.reciprocal_approx_fast(rec[:], av[:, D:D + 1])
                nc.vector.tensor_scalar(out=res[:, c, :], in0=av[:, 0:D],
                                        scalar1=rec[:, 0:1], scalar2=None,
                                        op0=OP.mult)

            def emit_slab(s):
                """scores+exp of slab s (if any) interleaved with AV of s-1."""
                ptc, ptp = pt_of(s), pt_of(s - 1)
                for gi in range(len(GROUPS)):
                    if s < NS:
                        score_group(s, ptc, gi)
                    if gi in (2, 5, 8, 11):
                        av_subtile(s - 1, ptp, (gi - 2) // 3)

            def emit_blend(c):
                nc.vector.copy_predicated(res[:, c, :],
                                          inv_u8[:, c:c + 1].to_broadcast([PART, D]),
                                          mvf[:])
                nc.sync.dma_start(out=out_re[:, c:c + 1, :], in_=res[:, c:c + 1, :])

            # ---- main loop: scores(s) interleaved with AV(s-1) ----
            sel_pass_init()
            sel_pass_full(0)
            blended = 0
            for s in range(1, NS + 1):
                emit_slab(s)
                if s == 1:
                    sel_pass_full(1)
                if s == 2:
                    sel_compact()
                if s == 3:
                    sel_pass_c(0)
                if s == 4:
                    sel_pass_c(1)
                if s == 5:
                    sel_pass_c(2)
                    sel_finish()
                if s >= 6:
                    # mask is ready; drain blends gradually (a burst would
                    # clog the DVE queue and starve the strip rings)
                    cap = min(4 * (s - 1), blended + 6)
                    while blended < cap:
                        emit_blend(blended)
                        blended += 1
            while blended < NT:
                emit_blend(blended)
                blended += 1

    nc.finalize()
    return nc


_CACHE = {}


def _get_nc(debug=False):
    key = bool(debug)
    if key not in _CACHE:
        _CACHE[key] = build(debug=key)
    return _CACHE[key]


def kernel(x, Wq, Wk, Wv, debug=False):
    nc = _get_nc(debug=debug)
    x = np.asarray(x, dtype=np.float32)
    in_maps = [
        {"x": np.ascontiguousarray(x[i]),
         "Wq": np.asarray(Wq, np.float32), "Wk": np.asarray(Wk, np.float32),
         "Wv": np.asarray(Wv, np.float32)}
        for i in range(B)
    ]
    last_err = None
    for _attempt in range(3):
        try:
            r = run_bass_kernel_spmd(nc, in_maps, core_ids=list(range(N_CORES)))
            out = np.stack([r.results[i]["out"] for i in range(B)]).astype(np.float32)
            break
        except Exception as e:  # transient axon RPC failures
            last_err = e
    else:
        raise last_err
    if debug:
        return out, r.results
    return out


# revision 68
# speedup vs baseline: 1.0053x; 1.0053x over previous
"""Trainium2 Bass kernel for nn_AttentionBlock_33724083208839 (sparse_attention).

Data-parallel over batch (8 batches -> 8 cores). Per core:
  1. chunked x load (all DMAs issued upfront), PE transpose -> xT f32;
     K projected in exact f32 (feeds selection), Q in f32; both also copied
     to bf16 (kTb via ACT, qTb via DVE) for the attention matmuls; V in f32
     -> bf16 [V|1] tiles.
  2. K_reduce via the exact CVaR identity sum_top_l = l*t + sum(relu(x-t)),
     t from Gaussian quantile + one Newton step on the exact count (f32 DVE).
  3. query selection: sqk = x @ (Wq @ K_reduce) exactly on PE; threshold =
     LQ-th largest of sqk via two full 128-ary counting passes on a
     partition-replicated copy, then sparse_gather (GPSIMD) compacts the
     ~10 in-interval values (num_found-masked: HW leaves garbage pads) and
     three cheap passes on the compacted set finish to f32 resolution.
  4. attention for all 4096 queries, 512-query slabs with a 1-slab lag and
     triple-buffered P^T tiles (so scores of slab s never wait on AV of
     slab s-2 releasing a buffer); x is loaded through a 16-chunk rolling
     window:
     scores^T on PE (bf16, 2-ktile PSUM strips) -> exp split between ACT
     (exact, scale=1/8, own 2-slot strip ring) and DVE (Schraudolph bitcast
     exp int16(A*s+B) -> bf16, own strip ring) -> P^T bf16 -> reversed AV:
     lhsT = P^T tile (stationary), rhs = [V|1] (65 moving cols) accumulated
     over 32 k-tiles, interleaved into the score-group stream, so outputs
     land directly in [query-partition, dv] layout; normalize by the
     ones-column denominator (DVE), blend non-selected rows to meanV
     (copy_predicated), DMA out per 128-query chunk.
"""
import sys

sys.path.insert(0, "/opt/trn_rl_repo")

import math
from statistics import NormalDist

import numpy as np

import concourse.bacc as bacc
import concourse.bass as bass
import concourse.bass_isa as bass_isa
import concourse.mybir as mybir
from concourse.tile import TileContext
from concourse.masks import make_identity
from concourse.bass_utils import run_bass_kernel_spmd

B, L, D = 8, 4096, 64
LQ = int((1.0 - 0.33) * L)  # 2744
PART = 128
NT = L // PART
NS = L // 512
N_CORES = 8

QFRAC = 1.0 - LQ / L
Z = NormalDist().inv_cdf(QFRAC)
PHI = math.exp(-Z * Z / 2.0) / math.sqrt(2.0 * math.pi)

f32 = mybir.dt.float32
f32r = mybir.dt.float32r
bf16 = mybir.dt.bfloat16
u8 = mybir.dt.uint8
i16 = mybir.dt.int16
i32 = mybir.dt.int32
AF = mybir.ActivationFunctionType
OP = mybir.AluOpType

N_PASS = 5
BOUND = 512.0

# Schraudolph exp for bf16 bit patterns: bf16_bits(exp(s/8)) ~= A*s + B.
# A = 128*log2(e)/8; B centers the piecewise-linear sawtooth (mean-unbiased)
# and adds +0.5 to compensate truncation in the float->int16 convert.
SCH_A = 128.0 * math.log2(math.e) / 8.0
SCH_B = 16256.0 + 0.5 - 128.0 * math.log2(1.0407)

GROUPS = [(g, 2) for g in range(0, NT, 2)]

# exp-engine split: selection runs on GPSIMD, so the DVE takes a fixed share
# of the exp strips (Schraudolph), spread through the slab so both engines
# drain the strip ring concurrently.
DVE_GROUPS = {2, 5, 8, 11, 14}
DVE_FROM_SLAB = 1
DVE_GROUPS_S0 = set()


def build(debug: bool = False):
    nc = bacc.Bacc("TRN2")
    x = nc.dram_tensor("x", [L, D], f32, kind="ExternalInput")
    wq = nc.dram_tensor("Wq", [D, D], f32, kind="ExternalInput")
    wk = nc.dram_tensor("Wk", [D, D], f32, kind="ExternalInput")
    wv = nc.dram_tensor("Wv", [D, D], f32, kind="ExternalInput")
    out = nc.dram_tensor("out", [L, D], f32, kind="ExternalOutput")
    scr_row = nc.dram_tensor("scr_row", [1, L], f32, kind="Internal")
    scr_cmp = nc.dram_tensor("scr_cmp", [1, 1024], f32, kind="Internal")
    scr_chi = nc.dram_tensor("scr_chi", [1, 16], f32, kind="Internal")
    dbg = {}
    if debug:
        for name, shape in [
            ("dbg_kr", [D, 1]), ("dbg_thr", [PART, 1]), ("dbg_sqk", [PART, NT]),
            ("dbg_mask", [PART, NT]), ("dbg_tk", [PART, 1]), ("dbg_cnt", [PART, 1]),
            ("dbg_sel", [PART, 8]), ("dbg_valrep", [PART, 64]),
        ]:
            dbg[name] = nc.dram_tensor(name, shape, f32, kind="ExternalOutput")

    x_re = x[:].rearrange("(c p) d -> p c d", p=PART)
    out_re = out[:].rearrange("(c p) d -> p c d", p=PART)

    with TileContext(nc) as tc, \
         tc.tile_pool(name="cst", bufs=1) as cst, \
         tc.tile_pool(name="big", bufs=1) as big, \
         tc.tile_pool(name="sc", bufs=1) as sc, \
         tc.tile_pool(name="mn", bufs=2) as mn:

        # ---- warm the exp activation table immediately ----
        warm = cst.tile([1, 8], f32)
        nc.vector.memset(warm[:], 0.0)
        warm2 = cst.tile([1, 8], f32)
        nc.scalar.activation(out=warm2[:], in_=warm[:], func=AF.Exp)

        # ---- constants ----
        ident = cst.tile([PART, PART], f32)
        make_identity(nc, ident[:])
        onesb = cst.tile([PART, 1], bf16)
        nc.vector.memset(onesb[:], 1.0)
        ones1x128 = cst.tile([1, PART], f32)
        nc.vector.memset(ones1x128[:], 1.0)
        iotc_i = cst.tile([16, 64], i32)
        nc.gpsimd.iota(iotc_i[:], pattern=[[16, 64]], base=0, channel_multiplier=1)
        iotc = cst.tile([16, 64], f32)
        nc.vector.tensor_copy(iotc[:], iotc_i[:])
        pidx1i = cst.tile([PART, 1], i32)
        nc.gpsimd.iota(pidx1i[:], pattern=[[1, 1]], base=1, channel_multiplier=1)
        pidx1 = cst.tile([PART, 1], f32)
        nc.vector.tensor_copy(pidx1[:], pidx1i[:])

        # ---- persistent tensors ----
        x_sb = big.tile([PART, 16, D], f32)
        xT32 = big.tile([D, L], f32)
        qTb = big.tile([D, L], bf16)
        kT32 = big.tile([D, L], f32)
        kTb = big.tile([D, L], bf16)
        vp = big.tile([PART, NT, D + 1], bf16)
        pt_a = big.tile([PART, NT, 512], bf16)
        pt_b = big.tile([PART, NT, 512], bf16)
        pt_c = big.tile([PART, NT, 512], bf16)
        res = big.tile([PART, NT, D], f32)
        mvf = big.tile([PART, D], f32)
        mask = big.tile([PART, NT], f32)
        inv_u8 = big.tile([PART, NT], u8)
        sqk = big.tile([PART, NT], f32)
        kr = big.tile([D, 1], f32)
        wvec = big.tile([D, 1], f32)
        sqk_rep = big.tile([PART, L], f32)
        cmp_rep = big.tile([PART, L], bf16)
        sqk16 = big.tile([16, 256], f32)
        valrep = big.tile([PART, 1024], f32)
        cmpc = big.tile([PART, 1024], bf16)

        # weights
        wq_s = cst.tile([D, D], f32)
        wk_s = cst.tile([D, D], f32)
        wv_s = cst.tile([D, D], f32)
        nc.sync.dma_start(out=wq_s[:], in_=wq[:])
        nc.sync.dma_start(out=wk_s[:], in_=wk[:])
        nc.sync.dma_start(out=wv_s[:], in_=wv[:])

        # =============== phase 1: load / project / slab-0 scores+exp ===============
        with tc.tile_pool(name="ps_xv", bufs=2, space="PSUM") as ps_xv, \
             tc.tile_pool(name="ps_pj", bufs=2, space="PSUM") as ps_pj, \
             tc.tile_pool(name="ps_s0", bufs=2, space="PSUM") as ps_s0:

            def load_tiles(c0, c1):
                for c in range(c0, c1):
                    pxt = ps_xv.tile([PART, PART], f32, tag="xv")
                    nc.tensor.transpose(out=pxt[0:D, :], in_=x_sb[:, c % 16, :],
                                        identity=ident[:])
                    nc.vector.tensor_copy(xT32[:, PART * c:PART * (c + 1)], pxt[0:D, :])
                if c0 + 16 < NT:
                    m = c0 % 16
                    nc.sync.dma_start(out=x_sb[:, m:m + (c1 - c0), :],
                                      in_=x_re[:, c0 + 16:c1 + 16, :])

            def proj_slab(s):
                sl = slice(512 * s, 512 * (s + 1))
                pk = ps_pj.tile([D, 512], f32, tag="pj")
                for h in range(2):
                    hs = slice(512 * s + 256 * h, 512 * s + 256 * (h + 1))
                    nc.tensor.matmul(out=pk[:, 256 * h:256 * (h + 1)], lhsT=wk_s[:],
                                     rhs=xT32[:, hs], start=True, stop=True)
                    nc.vector.tensor_copy(kT32[:, hs], pk[:, 256 * h:256 * (h + 1)])
                    nc.scalar.copy(kTb[:, hs], pk[:, 256 * h:256 * (h + 1)])
                pq = ps_pj.tile([D, 512], f32, tag="pj")
                nc.tensor.matmul(out=pq[:], lhsT=wq_s[:], rhs=xT32[:, sl],
                                 start=True, stop=True)
                nc.scalar.copy(qTb[:, sl], pq[:])

            def sg0(gi):
                g0, glen = GROUPS[gi]
                strip = ps_s0.tile([PART, 2, 512], f32, tag="s0")
                for i in range(glen):
                    j = g0 + i
                    nc.tensor.matmul(out=strip[:, i, :],
                                     lhsT=kTb[:, PART * j:PART * (j + 1)],
                                     rhs=qTb[:, 0:512], start=True, stop=True)
                if gi in DVE_GROUPS_S0:  # slab 0
                    nc.vector.tensor_scalar(
                        out=pt_a[:, g0:g0 + glen, :].bitcast(i16),
                        in0=strip[:, 0:glen, :], scalar1=SCH_A, scalar2=SCH_B,
                        op0=OP.mult, op1=OP.add)
                else:
                    nc.scalar.activation(out=pt_a[:, g0:g0 + glen, :],
                                         in_=strip[:, 0:glen, :], func=AF.Exp, scale=0.125)

            def proj_v(c0, c1):
                for c in range(c0, c1):
                    pv = ps_xv.tile([PART, PART], f32, tag="xv")
                    nc.tensor.matmul(out=pv[:, 0:D],
                                     lhsT=xT32[:, PART * c:PART * (c + 1)],
                                     rhs=wv_s[:], start=True, stop=True)
                    nc.vector.tensor_copy(vp[:, c, 0:D], pv[:, 0:D])

            for c0 in range(0, 16, 4):
                nc.sync.dma_start(out=x_sb[:, c0:c0 + 4, :], in_=x_re[:, c0:c0 + 4, :])
            load_tiles(0, 2)
            load_tiles(2, 4)
            proj_slab(0)
            proj_v(0, 4)
            sg0(0); sg0(1)
            load_tiles(4, 8)
            proj_slab(1)
            proj_v(4, 8)
            sg0(2); sg0(3)
            load_tiles(8, 12)
            load_tiles(12, 16)
            proj_slab(2); proj_v(8, 12); sg0(4); sg0(5)
            proj_slab(3); proj_v(12, 16); sg0(6); sg0(7)
            load_tiles(16, 20)
            load_tiles(20, 24)
            proj_slab(4); proj_v(16, 20); sg0(8); sg0(9)
            proj_slab(5); proj_v(20, 24); sg0(10); sg0(11)
            load_tiles(24, 28)
            load_tiles(28, 32)
            proj_slab(6); proj_v(24, 28); sg0(12); sg0(13)
            proj_slab(7); proj_v(28, 32)
            nc.vector.memset(vp[:, :, D:D + 1], 1.0)
            sg0(14)
            sg0(15)

        # ---- K_reduce (DVE only; channel = partition of kT32) ----
        bstats = sc.tile([D, 8, 6], f32, tag="bstats")
        for a in range(8):
            nc.vector.bn_stats(bstats[:, a, :], kT32[:, 512 * a:512 * (a + 1)])
        aggr = sc.tile([D, 2], f32, tag="aggr")
        nc.vector.bn_aggr(aggr[:], bstats[:])
        sig = sc.tile([D, 1], f32, tag="sig")
        nc.vector.memset(sig[:], 1.0)
        for _ in range(4):
            rs = sc.tile([D, 1], f32, tag="rs")
            nc.vector.reciprocal(rs[:], sig[:])
            nc.vector.tensor_tensor(out=rs[:], in0=rs[:], in1=aggr[:, 1:2], op=OP.mult)
            nc.vector.tensor_tensor(out=rs[:], in0=rs[:], in1=sig[:], op=OP.add)
            nc.vector.tensor_scalar_mul(sig[:], rs[:], 0.5)
        tk = sc.tile([D, 1], f32, tag="tk")
        nc.vector.tensor_scalar(out=tk[:], in0=sig[:], scalar1=float(Z),
                                scalar2=None, op0=OP.mult)
        nc.vector.tensor_tensor(out=tk[:], in0=tk[:], in1=aggr[:, 0:1], op=OP.add)
        cnt_c = sc.tile([D, 1], f32, tag="cnt_c")
        nc.vector.tensor_scalar(out=sqk_rep[0:D, :], in0=kT32[:], scalar1=tk[:, 0:1],
                                scalar2=None, op0=OP.is_gt, op1=OP.add,
                                accum_out=cnt_c[:])
        adj = sc.tile([D, 1], f32, tag="adj")
        nc.vector.tensor_scalar(out=adj[:], in0=cnt_c[:], scalar1=float(-LQ),
                                scalar2=1.0 / (L * PHI), op0=OP.add, op1=OP.mult)
        nc.vector.tensor_tensor(out=adj[:], in0=adj[:], in1=sig[:], op=OP.mult)
        t1 = sc.tile([D, 1], f32, tag="t1")
        nc.vector.tensor_tensor(out=t1[:], in0=tk[:], in1=adj[:], op=OP.add)
        nc.vector.tensor_scalar(out=sqk_rep[0:D, :], in0=kT32[:], scalar1=t1[:, 0:1],
                                scalar2=0.0, op0=OP.subtract, op1=OP.max)
        s1c = sc.tile([D, 1], f32, tag="s1c")
        nc.vector.tensor_reduce(out=s1c[:], in_=sqk_rep[0:D, :], axis=mybir.AxisListType.X,
                                op=OP.add)
        nc.vector.tensor_scalar(out=kr[:], in0=s1c[:], scalar1=1.0 / LQ,
                                scalar2=None, op0=OP.mult)
        nc.vector.tensor_tensor(out=kr[:], in0=kr[:], in1=t1[:], op=OP.add)

        emit_kred_a()

        # =============== phase 2: attention + selection ===============
        with tc.tile_pool(name="ps_strip", bufs=2, space="PSUM") as ps_strip, \
             tc.tile_pool(name="ps_dstrip", bufs=1, space="PSUM") as ps_dstrip, \
             tc.tile_pool(name="ps_av", bufs=2, space="PSUM") as ps_av:
            def mis_tile():
                return ps_av.tile([PART, PART], f32, tag="av", name="avm")

            def pt_of(s):
                return (pt_a, pt_b, pt_c)[s % 3]

            # ---- selection PE bits: wvec, sqk, replicate sqk ----
            pwt = mis_tile()
            nc.tensor.transpose(out=pwt[0:D, 0:D], in_=wq_s[:], identity=ident[0:D, 0:D])
            wqT = sc.tile([D, D], f32, tag="wqT")
            nc.vector.tensor_copy(wqT[:], pwt[0:D, 0:D])
            pw = mis_tile()
            nc.tensor.matmul(out=pw[0:D, 0:1], lhsT=wqT[:], rhs=kr[:],
                             start=True, stop=True)
            nc.vector.tensor_copy(wvec[:], pw[0:D, 0:1])
            psq = mis_tile()
            for c in range(NT):
                nc.tensor.matmul(out=psq[:, c:c + 1],
                                 lhsT=xT32[:, PART * c:PART * (c + 1)],
                                 rhs=wvec[:], start=True, stop=True)
            nc.vector.tensor_copy(sqk[:], psq[:, 0:NT])

            psqT = mis_tile()
            nc.tensor.transpose(out=psqT[0:NT, 0:PART], in_=sqk[:], identity=ident[:])
            sqkT = sc.tile([NT, PART], f32, tag="sqkT")
            nc.vector.tensor_copy(sqkT[:], psqT[0:NT, 0:PART])
            nc.sync.dma_start(out=scr_row[:], in_=sqkT[:])
            nc.sync.dma_start(out=sqk_rep[:], in_=scr_row[:].to_broadcast([PART, L]))

            if debug:
                nc.sync.dma_start(out=dbg["dbg_kr"][:], in_=kr[:])
                nc.sync.dma_start(out=dbg["dbg_sqk"][:], in_=sqk[:])
                nc.sync.dma_start(out=dbg["dbg_tk"][0:D, :], in_=t1[:])

            # ---- meanV on PE ----
            pmv = mis_tile()
            for c in range(NT):
                nc.tensor.matmul(out=pmv[0:D + 1, 0:1], lhsT=vp[:, c, :], rhs=onesb[:],
                                 start=(c == 0), stop=(c == NT - 1))
            mv_col = sc.tile([D, 1], f32, tag="mv_col")
            nc.vector.tensor_scalar_mul(mv_col[:], pmv[0:D, 0:1], 1.0 / L)
            pmvT = mis_tile()
            nc.tensor.transpose(out=pmvT[0:1, 0:D], in_=mv_col[:],
                                identity=ident[0:D, 0:D])
            mv_row = sc.tile([1, D], f32, tag="mv_row")
            nc.vector.tensor_copy(mv_row[:], pmvT[0:1, 0:D])
            pmvF = mis_tile()
            nc.tensor.matmul(out=pmvF[:, 0:D], lhsT=ones1x128[:], rhs=mv_row[:],
                             start=True, stop=True)
            nc.vector.tensor_copy(mvf[:], pmvF[:, 0:D])

            # ---- selection: two full 128-ary passes on the replicated sqk,
            # then sparse_gather compacts the ~10 in-interval values and three
            # cheap passes on the compacted set finish to f32 resolution ----
            sel_state = {}
            DLT1 = 2.0 * BOUND / 129.0
            DLT2 = DLT1 / 129.0        # interval width after pass 2
            DLTC = [DLT2 / 129.0, DLT2 / 129.0 ** 2, DLT2 / 129.0 ** 3]

            def sel_pass_init():
                lo = mn.tile([PART, 1], f32, tag="lo_a")
                nc.vector.memset(lo[:], -BOUND)
                sel_state["lo"] = lo

            def sel_pass_full(it):
                # thresholds t_p = lo + p*dlt; count(sqk > t_p) per partition
                lo = sel_state["lo"]
                dlt = DLT1 if it == 0 else DLT2
                tvec = mn.tile([PART, 1], f32, tag=f"tv{it % 2}")
                nc.vector.tensor_scalar(out=tvec[:], in0=pidx1[:], scalar1=float(dlt),
                                        scalar2=None, op0=OP.mult)
                nc.vector.tensor_tensor(out=tvec[:], in0=tvec[:], in1=lo[:], op=OP.add)
                cntq = mn.tile([PART, 1], f32, tag="cntq")
                nc.vector.tensor_scalar(out=cmp_rep[:], in0=sqk_rep[:],
                                        scalar1=tvec[:, 0:1], scalar2=None,
                                        op0=OP.is_gt, op1=OP.add, accum_out=cntq[:])
                sel = mn.tile([PART, 1], f32, tag="sel")
                nc.vector.tensor_scalar(out=sel[:], in0=cntq[:], scalar1=float(LQ),
                                        scalar2=None, op0=OP.is_ge)
                jsr = mn.tile([PART, 1], f32, tag="jsr")
                nc.gpsimd.partition_all_reduce(jsr[:], sel[:], channels=PART,
                                               reduce_op=bass_isa.ReduceOp.add)
                nlo = mn.tile([PART, 1], f32, tag=f"lo_{'b' if it % 2 == 0 else 'a'}")
                nc.vector.tensor_scalar(out=jsr[:], in0=jsr[:], scalar1=float(dlt),
                                        scalar2=None, op0=OP.mult)
                nc.vector.tensor_tensor(out=nlo[:], in0=lo[:], in1=jsr[:], op=OP.add)
                sel_state["lo"] = nlo

            def sel_compact():
                # threshold in (lo2, lo2 + DLT2]; c_hi = count(sqk > hi2) exact;
                # compact v' = sqk - lo2 for in-interval values via sparse_gather
                lo2 = sel_state["lo"]
                hi2 = mn.tile([PART, 1], f32, tag="hi2")
                nc.vector.tensor_scalar(out=hi2[:], in0=lo2[:], scalar1=float(DLT2),
                                        scalar2=None, op0=OP.add)
                nc.sync.dma_start(out=sqk16[:], in_=scr_row[0, :].rearrange(
                    "(f p) -> p f", p=16))
                # c_hi = count(sqk > hi2), counted on the [128, 32] per-query
                # tile + a channels=128 all-reduce (HW-proven path)
                j32 = sc.tile([PART, NT], f32, tag="j32")
                chi = sc.tile([PART, 1], f32, tag="chi")
                nc.vector.tensor_scalar(out=j32[:], in0=sqk[:],
                                        scalar1=hi2[:, 0:1], scalar2=None,
                                        op0=OP.is_gt, op1=OP.add, accum_out=chi[:])
                nc.gpsimd.partition_all_reduce(chi[:], chi[:], channels=PART,
                                               reduce_op=bass_isa.ReduceOp.add)
                rvec = sc.tile([PART, 1], f32, tag="rvec")
                nc.vector.tensor_scalar(out=rvec[:], in0=chi[:], scalar1=-1.0,
                                        scalar2=float(LQ), op0=OP.mult, op1=OP.add)
                sel_state["rvec"] = rvec
                # Tv = (sqk-lo2)*b + (b-1) with b = (sqk <= hi2): in-interval ->
                # positive v', others -> negative (sparse_gather keeps >= 0)
                a16 = sc.tile([16, 256], f32, tag="a16")
                nc.vector.tensor_scalar(out=a16[:], in0=sqk16[:],
                                        scalar1=lo2[0:16, 0:1], scalar2=None,
                                        op0=OP.subtract)
                b16 = sc.tile([16, 256], f32, tag="b16")
                nc.vector.tensor_scalar(out=b16[:], in0=sqk16[:],
                                        scalar1=hi2[0:16, 0:1], scalar2=None,
                                        op0=OP.is_le)
                tv16 = sc.tile([16, 256], f32, tag="tv16")
                nc.vector.tensor_tensor(out=tv16[:], in0=a16[:], in1=b16[:],
                                        op=OP.mult)
                nc.vector.tensor_scalar(out=b16[:], in0=b16[:], scalar1=1.0,
                                        scalar2=None, op0=OP.subtract)
                nc.vector.tensor_tensor(out=tv16[:], in0=tv16[:], in1=b16[:],
                                        op=OP.add)
                valc = sc.tile([16, 64], f32, tag="valc")
                nc.vector.memset(valc[:], -1.0)
                nfound = sc.tile([1, 1], mybir.dt.uint32, tag="nfound")
                nc.gpsimd.sparse_gather(valc[:], tv16[:], num_found=nfound[:])
                # HW sparse_gather leaves garbage beyond num_found: mask pads
                nf32 = sc.tile([1, 1], f32, tag="nf32")
                nc.vector.tensor_copy(nf32[:], nfound[:])
                nc.sync.dma_start(out=scr_chi[0:1, 0:1], in_=nf32[:])
                nfb = sc.tile([16, 1], f32, tag="nfb")
                nc.sync.dma_start(out=nfb[:],
                                  in_=scr_chi[0:1, 0:1].to_broadcast([16, 1]))
                vmask = sc.tile([16, 64], f32, tag="vmask")
                nc.vector.tensor_scalar(out=vmask[:], in0=iotc[:],
                                        scalar1=nfb[:, 0:1], scalar2=None,
                                        op0=OP.is_lt)
                nc.vector.tensor_tensor(out=valc[:], in0=valc[:], in1=vmask[:],
                                        op=OP.mult)
                nc.vector.tensor_scalar(out=vmask[:], in0=vmask[:], scalar1=1.0,
                                        scalar2=None, op0=OP.subtract)
                nc.vector.tensor_tensor(out=valc[:], in0=valc[:], in1=vmask[:],
                                        op=OP.add)
                nc.sync.dma_start(out=scr_cmp[0, :].rearrange("(f p) -> p f", p=16),
                                  in_=valc[:])
                nc.sync.dma_start(out=valrep[:],
                                  in_=scr_cmp[:].to_broadcast([PART, 1024]))
                loc = mn.tile([PART, 1], f32, tag="loc_a")
                nc.vector.memset(loc[:], 0.0)
                sel_state["loc"] = loc

            def sel_pass_c(it):
                loc, rvec = sel_state["loc"], sel_state["rvec"]
                dlt = DLTC[it]
                tvec = mn.tile([PART, 1], f32, tag=f"tvc{it % 2}")
                nc.vector.tensor_scalar(out=tvec[:], in0=pidx1[:], scalar1=float(dlt),
                                        scalar2=None, op0=OP.mult)
                nc.vector.tensor_tensor(out=tvec[:], in0=tvec[:], in1=loc[:], op=OP.add)
                cntq = mn.tile([PART, 1], f32, tag="cntqc")
                nc.vector.tensor_scalar(out=cmpc[:], in0=valrep[:],
                                        scalar1=tvec[:, 0:1], scalar2=None,
                                        op0=OP.is_gt, op1=OP.add, accum_out=cntq[:])
                sel = mn.tile([PART, 1], f32, tag="selc")
                nc.vector.tensor_scalar(out=sel[:], in0=cntq[:],
                                        scalar1=rvec[:, 0:1], scalar2=None,
                                        op0=OP.is_ge)
                jsr = mn.tile([PART, 1], f32, tag="jsrc")
                nc.gpsimd.partition_all_reduce(jsr[:], sel[:], channels=PART,
                                               reduce_op=bass_isa.ReduceOp.add)
                nlo = mn.tile([PART, 1], f32, tag=f"loc_{'b' if it % 2 == 0 else 'a'}")
                nc.vector.tensor_scalar(out=jsr[:], in0=jsr[:], scalar1=float(dlt),
                                        scalar2=None, op0=OP.mult)
                nc.vector.tensor_tensor(out=nlo[:], in0=loc[:], in1=jsr[:], op=OP.add)
                sel_state["loc"] = nlo

            def sel_finish():
                lo2, loc = sel_state["lo"], sel_state["loc"]
                if debug:
                    dsel = mn.tile([PART, 8], f32, tag="dsel")
                    nc.vector.tensor_copy(dsel[:, 0:1], sel_state["rvec"][:])
                    nc.vector.tensor_copy(dsel[:, 1:2], lo2[:])
                    nc.vector.tensor_copy(dsel[:, 2:3], loc[:])
                    nc.sync.dma_start(out=dbg["dbg_sel"][:], in_=dsel[:])
                    nc.sync.dma_start(out=dbg["dbg_valrep"][:], in_=valrep[:, 0:64])
                v128 = mn.tile([PART, NT], f32, tag="v128")
                nc.vector.tensor_scalar(out=v128[:], in0=sqk[:],
                                        scalar1=lo2[:, 0:1], scalar2=None,
                                        op0=OP.subtract)
                nc.vector.tensor_scalar(out=mask[:], in0=v128[:],
                                        scalar1=loc[:, 0:1], scalar2=None,
                                        op0=OP.is_gt)
                minv = mn.tile([PART, NT], f32, tag="minv")
                nc.vector.tensor_scalar(out=minv[:], in0=mask[:], scalar1=-1.0,
                                        scalar2=1.0, op0=OP.mult, op1=OP.add)
                nc.vector.tensor_copy(inv_u8[:], minv[:])
                if debug:
                    thrd = mn.tile([PART, 1], f32, tag="thrd")
                    nc.vector.tensor_tensor(out=thrd[:], in0=lo2[:], in1=loc[:],
                                            op=OP.add)
                    nc.sync.dma_start(out=dbg["dbg_mask"][:], in_=mask[:])
                    nc.sync.dma_start(out=dbg["dbg_thr"][:], in_=thrd[:])
                    cntf = mn.tile([PART, 1], f32, tag="cntf")
                    cmpf = mn.tile([PART, NT], f32, tag="cmpf")
                    nc.vector.tensor_scalar(out=cmpf[:], in0=mask[:], scalar1=1.0,
                                            scalar2=None, op0=OP.mult, op1=OP.add,
                                            accum_out=cntf[:])
                    nc.sync.dma_start(out=dbg["dbg_cnt"][:], in_=cntf[:])

            # ---- attention slab machinery ----
            def score_group(s, ptc, gi):
                g0, glen = GROUPS[gi]
                is_dve = s >= DVE_FROM_SLAB and gi in DVE_GROUPS
                if is_dve:
                    strip = ps_dstrip.tile([PART, 2, 512], f32, tag="dstrip")
                else:
                    strip = ps_strip.tile([PART, 2, 512], f32, tag="strip")
                for i in range(glen):
                    j = g0 + i
                    nc.tensor.matmul(out=strip[:, i, :],
                                     lhsT=kTb[:, PART * j:PART * (j + 1)],
                                     rhs=qTb[:, 512 * s:512 * (s + 1)],
                                     start=True, stop=True)
                if is_dve:
                    nc.vector.tensor_scalar(
                        out=ptc[:, g0:g0 + glen, :].bitcast(i16),
                        in0=strip[:, 0:glen, :], scalar1=SCH_A, scalar2=SCH_B,
                        op0=OP.mult, op1=OP.add)
                else:
                    nc.scalar.activation(out=ptc[:, g0:g0 + glen, :],
                                         in_=strip[:, 0:glen, :], func=AF.Exp,
                                         scale=0.125)

            def av_subtile(s, ptp, u):
                c = 4 * s + u
                av = mis_tile()
                for j in range(NT):
                    nc.tensor.matmul(out=av[:, 0:D + 1],
                                     lhsT=ptp[:, j, PART * u:PART * (u + 1)],
                                     rhs=vp[:, j, :],
                                     start=(j == 0), stop=(j == NT - 1))
                rec = mn.tile([PART, 1], f32, tag="rec")
                nc.vector.reciprocal_approx_fast(rec[:], av[:, D:D + 1])
                nc.vector.tensor_scalar(out=res[:, c, :], in0=av[:, 0:D],
                                        scalar1=rec[:, 0:1], scalar2=None,
                                        op0=OP.mult)

            def emit_slab(s):
                """scores+exp of slab s (if any) interleaved with AV of s-1."""
                ptc, ptp = pt_of(s), pt_of(s - 1)
                for gi in range(len(GROUPS)):
                    if s < NS:
                        score_group(s, ptc, gi)
                    if gi in (2, 5, 8, 11):
                        av_subtile(s - 1, ptp, (gi - 2) // 3)

            def emit_blend(c):
                nc.vector.copy_predicated(res[:, c, :],
                                          inv_u8[:, c:c + 1].to_broadcast([PART, D]),
                                          mvf[:])
                nc.sync.dma_start(out=out_re[:, c:c + 1, :], in_=res[:, c:c + 1, :])

            # ---- main loop: scores(s) interleaved with AV(s-1) ----
            sel_pass_init()
            sel_pass_full(0)
            blended = 0
            for s in range(1, NS + 1):
                emit_slab(s)
                if s == 1:
                    sel_pass_full(1)
                if s == 2:
                    sel_compact()
                if s == 3:
                    sel_pass_c(0)
                if s == 4:
                    sel_pass_c(1)
                if s == 5:
                    sel_pass_c(2)
                    sel_finish()
                if s >= 6:
                    # mask is ready; drain blends gradually (a burst would
                    # clog the DVE queue and starve the strip rings)
                    cap = min(4 * (s - 1), blended + 6)
                    while blended < cap:
                        emit_blend(blended)
                        blended += 1
            while blended < NT:
                emit_blend(blended)
                blended += 1

    nc.finalize()
    return nc


_CACHE = {}


def _get_nc(debug=False):
    key = bool(debug)
    if key not in _CACHE:
        _CACHE[key] = build(debug=key)
    return _CACHE[key]


def kernel(x, Wq, Wk, Wv, debug=False):
    nc = _get_nc(debug=debug)
    x = np.asarray(x, dtype=np.float32)
    in_maps = [
        {"x": np.ascontiguousarray(x[i]),
         "Wq": np.asarray(Wq, np.float32), "Wk": np.asarray(Wk, np.float32),
         "Wv": np.asarray(Wv, np.float32)}
        for i in range(B)
    ]
    last_err = None
    for _attempt in range(3):
        try:
            r = run_bass_kernel_spmd(nc, in_maps, core_ids=list(range(N_CORES)))
            out = np.stack([r.results[i]["out"] for i in range(B)]).astype(np.float32)
            break
        except Exception as e:  # transient axon RPC failures
            last_err = e
    else:
        raise last_err
    if debug:
        return out, r.results
    return out
